# revision 9
# baseline (speedup 1.0000x reference)
"""CLFormer Trainium2 kernel: 3x (linear attention + FFN) + 1x1 conv head.

Data-parallel over batch: 32 batches -> 8 NeuronCores x 4 batches.
Per (batch, block), everything stays on-chip:
  h (D-major, f32r)  --ACT exp-->  E (bf16) + kdenom (f32, accum)
  PE transposes h, E into S-major (bf16) for the ctx matmul
  ctx[d,e] = sum_s E[s,d] h[s,e]   (PE, 64 chunk matmuls, f32 PSUM)
  ctx normalized by 1/kdenom per row, assembled block-diagonal (bf16)
  qdenom^T via per-chunk PE matmuls (E chunk stationary x ones4)
  att (S-major) = E_chunk^T @ ctx_bd, normalized by broadcast 1/qdenom
  att transposed back to D-major (f32r) -> FFN (f32r matmuls + ACT gelu)
  h' = att + gelu(FFN2)  (DVE)
Then out = Wf @ h + bf (PE + DVE bias), DMA'd out.
"""
import sys

sys.path.insert(0, "/opt/trn_rl_repo")

import numpy as np
import ml_dtypes

import concourse.bass as bass
import concourse.tile as tile
from concourse import mybir
from concourse.bass_utils import run_bass_kernel_spmd
from concourse.vector_clock import ScopedClock

# ----------------------------------------------------------------------------
# Workaround for this walrus build: at most ONE attached sem wait per
# instruction. Extra waits are re-emitted on same-engine NOP carriers placed
# immediately before the instruction (sequential waits AND together).
# ----------------------------------------------------------------------------
_MAXW = 1
_orig_add_instruction = tile.TileContext._add_instruction


def _split_waits(nc, inst):
    si = inst.sync_info
    if si is None:
        return []
    waits = list(si.on_wait)
    if len(waits) <= _MAXW:
        return []
    keep, extra = waits[-_MAXW:], waits[:-_MAXW]
    carriers = []
    for w in extra:
        nop = mybir.InstNoOp(name=nc.get_next_instruction_name(), ins=[], outs=[])
        nop.engine = inst.engine
        nop.sync_info = mybir.SyncInfo(on_wait=[w], on_update=[])
        if inst.debug is not None:
            nop.debug = inst.debug
        carriers.append(nop)
    inst.sync_info = mybir.SyncInfo(on_wait=keep, on_update=list(si.on_update))
    return carriers


def _patched_add_instruction(self, inst):
    if (
        inst.engine is not None
        and inst.engine != mybir.EngineType.Unassigned
        and inst.sync_info is not None
        and len(inst.sync_info.on_wait) > _MAXW
    ):
        for nop in _split_waits(self.nc, inst):
            _orig_add_instruction(self, nop)
    return _orig_add_instruction(self, inst)


def _patched_drain_and_barrier(self, tick_clock, wait_clock):
    nc = self.nc
    drain_inst = nc.sync.drain()
    wait_clock.add_sem_waits(
        drain_inst.ins, ScopedClock({None: tick_clock.global_clock})
    )
    si = drain_inst.ins.sync_info
    waits = list(si.on_wait) if si is not None else []
    if len(waits) > _MAXW:
        drain_inst.ins.sync_info = mybir.SyncInfo(on_wait=[], on_update=[])
        for i in range(0, len(waits), _MAXW):
            nop = nc.sync.nop(nofuse=True)
            nop.ins.sync_info = mybir.SyncInfo(
                on_wait=waits[i : i + _MAXW], on_update=[]
            )
    nc.all_engine_barrier()
    assert self.sems is not None
    popped = nc._tile_sem_poison_stack.pop()
    assert popped is self._sem_poison
    nc.clear_and_free_semaphores(list(self.sems.allocated().values()))
    nc.all_engine_barrier()


tile.TileContext._add_instruction = _patched_add_instruction
tile.TileContext._drain_and_barrier = _patched_drain_and_barrier

# ----------------------------------------------------------------------------
# Problem constants (hardcoded per contract)
# ----------------------------------------------------------------------------
CFG = dict(B=32, D=128, S=8192, H=4, NBLK=3, DOUT=64, N_CORES=8)

f32 = mybir.dt.float32
f32r = mybir.dt.float32r
bf16 = mybir.dt.bfloat16
f16 = mybir.dt.float16

AFT = mybir.ActivationFunctionType


def _act_recip(nc, out_ap, in_ap):
    """ACT Reciprocal via direct emission (bass guards it, but measured
    accuracy on positive well-scaled inputs is ~1e-5 rel)."""
    eng = nc.scalar
    ins = [eng.lower_ap(in_ap)]
    for arg in (0.0, 1.0, 0.0):
        ins.append(mybir.ImmediateValue(dtype=mybir.dt.float32, value=arg))
    return eng.add_instruction(
        mybir.InstActivation(
            name=nc.get_next_instruction_name(),
            func=AFT.Reciprocal,
            ins=ins,
            outs=[eng.lower_ap(out_ap)],
        )
    )


def build_program():
    B, D, S, H, NBLK, DOUT, N_CORES = (
        CFG["B"], CFG["D"], CFG["S"], CFG["H"], CFG["NBLK"], CFG["DOUT"],
        CFG["N_CORES"],
    )
    XB = B // N_CORES          # batches per core
    DH = D // H                # head dim (32)
    NC128 = S // 128           # 128-col chunks (64)
    NC512 = S // 512           # 512-col chunks (16)

    nc = bass.Bass("TRN2", target_bir_lowering=False, debug=False,
                   num_devices=N_CORES)

    # ---- DRAM I/O ----
    x_d = nc.dram_tensor("x", [XB, D, S], f32r, kind="ExternalInput").ap()
    pe_d = nc.dram_tensor("pe_t", [D, S], f16, kind="ExternalInput").ap()
    w1_d = nc.dram_tensor("w1t", [NBLK, D, D], f32r, kind="ExternalInput").ap()
    b1_d = nc.dram_tensor("b1c", [NBLK, D, 1], f32, kind="ExternalInput").ap()
    w2_d = nc.dram_tensor("w2t", [NBLK, D, D], f32r, kind="ExternalInput").ap()
    b2_d = nc.dram_tensor("b2c", [NBLK, D, 1], f32, kind="ExternalInput").ap()
    wf_d = nc.dram_tensor("wft", [D, DOUT], f32r, kind="ExternalInput").ap()
    bf_d = nc.dram_tensor("bfc", [DOUT, 1], f32, kind="ExternalInput").ap()
    on4_d = nc.dram_tensor("ones4", [D, H], bf16, kind="ExternalInput").ap()
    idr_d = nc.dram_tensor("ident_r", [128, 128], f32r, kind="ExternalInput").ap()
    idb_d = nc.dram_tensor("ident_b", [128, 128], bf16, kind="ExternalInput").ap()
    y_d = nc.dram_tensor("y", [XB, DOUT, S], f32, kind="ExternalOutput").ap()

    dumps = {}
    if CFG.get("DUMP"):
        for nm, shape, dt in [
            ("d_h0", [D, S], f32), ("d_E", [D, S], f32),
            ("d_kden", [D, 1], f32), ("d_hsm", [128, S], f32),
            ("d_esm", [128, S], f32), ("d_ctx", [128, 128], f32),
            ("d_ctxbd", [128, 128], f32), ("d_qd", [128, 4 * S // 128], f32),
            ("d_rqT", [128, 4 * S // 128], f32), ("d_asm", [128, 512], f32),
            ("d_adm", [128, 512], f32), ("d_f1", [128, 512], f32),
            ("d_f2g", [128, 512], f32), ("d_h1", [D, S], f32),
        ]:
            dumps[nm] = nc.dram_tensor(nm, shape, dt, kind="ExternalOutput").ap()

    with tile.TileContext(nc) as tc:
        import contextlib
        ctxmgr = contextlib.ExitStack()
        with ctxmgr:
            singles = ctxmgr.enter_context(tc.tile_pool(name="singles", bufs=1))
            hpool = ctxmgr.enter_context(tc.tile_pool(name="hpool", bufs=2))
            epool = ctxmgr.enter_context(tc.tile_pool(name="epool", bufs=1))
            smpool = ctxmgr.enter_context(tc.tile_pool(name="smpool", bufs=1))
            ringp = ctxmgr.enter_context(tc.tile_pool(name="rings", bufs=4))
            ring2 = ctxmgr.enter_context(tc.tile_pool(name="rings2", bufs=2))
            smallp = ctxmgr.enter_context(tc.tile_pool(name="smallp", bufs=2))
            outp = ctxmgr.enter_context(tc.tile_pool(name="outp", bufs=2))
            ps_tp = ctxmgr.enter_context(
                tc.tile_pool(name="ps_tp", bufs=2, space="PSUM"))
            ps_ctx = ctxmgr.enter_context(
                tc.tile_pool(name="ps_ctx", bufs=1, space="PSUM"))
            ps_qd = ctxmgr.enter_context(
                tc.tile_pool(name="ps_qd", bufs=1, space="PSUM"))
            ps_att = ctxmgr.enter_context(
                tc.tile_pool(name="ps_att", bufs=2, space="PSUM"))
            ps_ffn = ctxmgr.enter_context(
                tc.tile_pool(name="ps_ffn", bufs=2, space="PSUM"))

            # ---- constants ----
            peT = singles.tile([D, S], f16)
            nc.gpsimd.dma_start(out=peT[:], in_=pe_d[:])
            w1t = [singles.tile([D, D], f32r, name=f"w1t{i}", tag=f"w1t{i}") for i in range(NBLK)]
            w2t = [singles.tile([D, D], f32r, name=f"w2t{i}", tag=f"w2t{i}") for i in range(NBLK)]
            b1c = [singles.tile([D, 1], f32, name=f"b1c{i}", tag=f"b1c{i}") for i in range(NBLK)]
            b2c = [singles.tile([D, 1], f32, name=f"b2c{i}", tag=f"b2c{i}") for i in range(NBLK)]
            for i in range(NBLK):
                nc.gpsimd.dma_start(out=w1t[i][:], in_=w1_d[i])
                nc.gpsimd.dma_start(out=w2t[i][:], in_=w2_d[i])
                nc.gpsimd.dma_start(out=b1c[i][:], in_=b1_d[i])
                nc.gpsimd.dma_start(out=b2c[i][:], in_=b2_d[i])
            wft = singles.tile([D, DOUT], f32r)
            bfc = singles.tile([DOUT, 1], f32)
            on4 = singles.tile([D, H], bf16)
            idr = singles.tile([128, 128], f32r)
            idb = singles.tile([128, 128], bf16)
            nc.gpsimd.dma_start(out=wft[:], in_=wf_d[:])
            nc.gpsimd.dma_start(out=bfc[:], in_=bf_d[:])
            nc.gpsimd.dma_start(out=on4[:], in_=on4_d[:])
            nc.gpsimd.dma_start(out=idr[:], in_=idr_d[:])
            nc.gpsimd.dma_start(out=idb[:], in_=idb_d[:])

            dbgp = ctxmgr.enter_context(tc.tile_pool(name="dbgp", bufs=2))

            def dump(nm, src_ap, cast=False):
                if nm not in dumps:
                    return
                if cast:
                    shp = list(src_ap.shape)
                    stg = dbgp.tile(shp, f32, name=f"stg_{nm}", tag="stg")
                    nc.vector.tensor_copy(stg[:], src_ap)
                    nc.gpsimd.dma_start(out=dumps[nm][:], in_=stg[:])
                else:
                    nc.gpsimd.dma_start(out=dumps[nm][:], in_=src_ap)

            for b in range(XB):
                # ---- load x[b] and add positional encoding ----
                h = hpool.tile([D, S], f32r, tag="h")
                CH = min(2048, S)
                for k in range(S // CH):
                    sl = slice(CH * k, CH * (k + 1))
                    nc.gpsimd.dma_start(out=h[:, sl], in_=x_d[b, :, sl])
                for k in range(S // CH):
                    sl = slice(CH * k, CH * (k + 1))
                    nc.vector.tensor_add(h[:, sl], h[:, sl], peT[:, sl])

                for blk in range(NBLK):
                    dbg = CFG.get("DUMP") and b == 0 and blk == 0
                    if dbg:
                        dump("d_h0", h[:], cast=True)
                    # ======== Phase A ========
                    E = epool.tile([D, S], bf16, tag="E")
                    kden = smallp.tile([D, 1], f32, tag="kden")
                    nc.scalar.activation(E[:], h[:], AFT.Exp,
                                         accum_out=kden[:])
                    if dbg:
                        dump("d_E", E[:], cast=True)
                        dump("d_kden", kden[:])

                    h_sm = smpool.tile([128, S], bf16, tag="h_sm")
                    E_sm = smpool.tile([128, S], bf16, tag="E_sm")
                    # h transposes: 4 chunks of 128 -> one (128,512) f32r psum
                    for g in range(S // 512):
                        tp = ps_tp.tile([128, 512], f32r, tag="tp")
                        for j in range(4):
                            c0 = 512 * g + 128 * j
                            nc.tensor.transpose(
                                tp[:, 128 * j:128 * (j + 1)],
                                h[:, c0:c0 + 128], idr[:])
                        nc.vector.tensor_copy(
                            h_sm[:, 512 * g:512 * (g + 1)], tp[:])
                    # E transposes (bf16): 8 chunks -> (128,1024) bf16 psum
                    for g in range(S // 1024):
                        tp = ps_tp.tile([128, 1024], bf16, tag="tp")
                        for j in range(8):
                            c0 = 1024 * g + 128 * j
                            nc.tensor.transpose(
                                tp[:, 128 * j:128 * (j + 1)],
                                E[:, c0:c0 + 128], idb[:])
                        nc.vector.tensor_copy(
                            E_sm[:, 1024 * g:1024 * (g + 1)], tp[:])
                    if dbg:
                        dump("d_hsm", h_sm[:], cast=True)
                        dump("d_esm", E_sm[:], cast=True)

                    # ctx accumulation over all chunks
                    ctx_ps = ps_ctx.tile([128, 128], f32, tag="ctx")
                    for c in range(NC128):
                        sl = slice(128 * c, 128 * (c + 1))
                        nc.tensor.matmul(ctx_ps[:], E_sm[:, sl], h_sm[:, sl],
                                         start=(c == 0), stop=(c == NC128 - 1))

                    # normalize ctx rows by 1/kdenom, assemble block-diagonal
                    rk = smallp.tile([D, 1], f32, tag="rk")
                    _act_recip(nc, rk[:], kden[:])
                    ctx_bd = smallp.tile([128, 128], bf16, tag="ctx_bd")
                    nc.vector.memset(ctx_bd[:], 0.0)
                    for hh in range(H):
                        sl = slice(DH * hh, DH * (hh + 1))
                        nc.vector.tensor_scalar_mul(
                            ctx_bd[sl, sl], ctx_ps[sl, sl], rk[sl])
                    if dbg:
                        dump("d_ctx", ctx_ps[:], cast=True)
                        dump("d_ctxbd", ctx_bd[:], cast=True)

                    # ======== Phase B ========
                    # qdenom^T: per chunk (E chunk stationary) x ones4
                    qd_ps = ps_qd.tile([128, 4 * NC128], f32, tag="qd")
                    for c in range(NC128):
                        nc.tensor.matmul(
                            qd_ps[:, 4 * c:4 * (c + 1)],
                            E[:, 128 * c:128 * (c + 1)], on4[:],
                            start=True, stop=True)
                    rqT = smallp.tile([128, 4 * NC128], f32, tag="rqT")
                    if dbg:
                        dump("d_qd", qd_ps[:], cast=True)
                    _act_recip(nc, rqT[:], qd_ps[:])
                    if dbg:
                        dump("d_rqT", rqT[:])

                    for n in range(NC512):
                        # att (S-major) for 4 chunks
                        att_ps = ps_att.tile([128, 512], f32, tag="att")
                        for j in range(4):
                            c = 4 * n + j
                            nc.tensor.matmul(
                                att_ps[:, 128 * j:128 * (j + 1)],
                                E[:, 128 * c:128 * (c + 1)], ctx_bd[:],
                                start=True, stop=True)
                        # normalize by broadcast 1/qdenom and evac (bf16)
                        asm = ring2.tile([128, 512], bf16, tag="asm")
                        base = rqT[:, 16 * n:16 * n + 16]
                        rq_b = bass.AP(tensor=base.tensor, offset=base.offset,
                                       ap=[base.ap[0], [4, 4], [1, 4], [0, DH]])
                        nc.vector.tensor_tensor(out=asm[:], in0=att_ps[:],
                                                in1=rq_b,
                                                op=mybir.AluOpType.mult)
                        # transpose back to D-major
                        tp = ps_tp.tile([128, 512], bf16, tag="tp")
                        for j in range(4):
                            nc.tensor.transpose(
                                tp[:, 128 * j:128 * (j + 1)],
                                asm[:, 128 * j:128 * (j + 1)], idb[:])
                        adm = ringp.tile([128, 512], f32r, tag="adm")
                        nc.vector.tensor_copy(adm[:], tp[:])
                        if dbg and n == 0:
                            dump("d_asm", asm[:], cast=True)
                            dump("d_adm", adm[:], cast=True)

                        # FFN on this 512-column chunk
                        f1ps = ps_ffn.tile([128, 512], f32, tag="ffn")
                        nc.tensor.matmul(f1ps[:], w1t[blk][:], adm[:],
                                         start=True, stop=True)
                        f1 = ring2.tile([128, 512], f32r, tag="f1")
                        nc.scalar.activation(f1[:], f1ps[:], AFT.Gelu,
                                             bias=b1c[blk][:], scale=1.0)
                        f2ps = ps_ffn.tile([128, 512], f32, tag="ffn")
                        nc.tensor.matmul(f2ps[:], w2t[blk][:], f1[:],
                                         start=True, stop=True)
                        f2g = ring2.tile([128, 512], f32, tag="f2g")
                        nc.scalar.activation(f2g[:], f2ps[:], AFT.Gelu,
                                             bias=b2c[blk][:], scale=1.0)
                        if dbg and n == 0:
                            dump("d_f1", f1[:], cast=True)
                            dump("d_f2g", f2g[:])
                        # residual -> h (in place)
                        sl = slice(512 * n, 512 * (n + 1))
                        nc.vector.tensor_add(h[:, sl], adm[:], f2g[:])
                    if dbg:
                        dump("d_h1", h[:], cast=True)

                # ---- conv head: y[b] = Wf @ h + bf ----
                CHO = min(2048, S)
                for k in range(S // CHO):
                    osb = outp.tile([DOUT, CHO], f32, tag="osb")
                    for j in range(CHO // 512):
                        n0 = CHO * k + 512 * j
                        cps = ps_ffn.tile([DOUT, 512], f32, tag="ffn")
                        nc.tensor.matmul(cps[:], wft[:], h[:, n0:n0 + 512],
                                         start=True, stop=True)
                        nc.vector.tensor_scalar_add(
                            osb[:, 512 * j:512 * (j + 1)], cps[:], bfc[:])
                    sl = slice(CHO * k, CHO * (k + 1))
                    nc.gpsimd.dma_start(out=y_d[b, :, sl], in_=osb[:])

    return nc


# ----------------------------------------------------------------------------
# Host-side wrapper
# ----------------------------------------------------------------------------
_prog_cache = {}
_last_results = None


def _host_consts():
    D, S, H = CFG["D"], CFG["S"], CFG["H"]
    pos = np.arange(S, dtype=np.float32)[:, None]
    div = np.exp(np.arange(0, D, 2, dtype=np.float32)
                 * (-np.log(np.float32(10000.0)) / D)).astype(np.float32)
    ang = pos * div[None, :]
    pe = np.stack([np.sin(ang), np.cos(ang)], axis=-1).reshape(S, D)
    peT = np.ascontiguousarray(pe.T).astype(ml_dtypes.float16
                                            if False else np.float16)
    ones4 = np.zeros((D, H), dtype=ml_dtypes.bfloat16)
    for d in range(D):
        ones4[d, d // (D // H)] = 1
    ident = np.eye(128, dtype=np.float32)
    return peT, ones4, ident


def kernel(x, W1, b1, W2, b2, Wf, bf):
    B, D, S, NBLK, DOUT, N_CORES = (
        CFG["B"], CFG["D"], CFG["S"], CFG["NBLK"], CFG["DOUT"], CFG["N_CORES"])
    XB = B // N_CORES

    x = np.asarray(x, dtype=np.float32)
    W1 = np.asarray(W1, dtype=np.float32)
    b1 = np.asarray(b1, dtype=np.float32)
    W2 = np.asarray(W2, dtype=np.float32)
    b2 = np.asarray(b2, dtype=np.float32)
    Wf = np.asarray(Wf, dtype=np.float32)
    bf = np.asarray(bf, dtype=np.float32)

    key = "prog"
    if key not in _prog_cache:
        _prog_cache[key] = build_program()
    nc = _prog_cache[key]

    peT, ones4, ident = _host_consts()
    w1t = np.ascontiguousarray(np.transpose(W1, (0, 2, 1)))
    w2t = np.ascontiguousarray(np.transpose(W2, (0, 2, 1)))
    consts = {
        "pe_t": peT,
        "w1t": w1t,
        "b1c": b1.reshape(NBLK, D, 1),
        "w2t": w2t,
        "b2c": b2.reshape(NBLK, D, 1),
        "wft": np.ascontiguousarray(Wf.T),
        "bfc": bf.reshape(DOUT, 1),
        "ones4": ones4,
        "ident_r": ident,
        "ident_b": ident.astype(ml_dtypes.bfloat16),
    }
    in_maps = []
    for c in range(N_CORES):
        m = dict(consts)
        m["x"] = np.ascontiguousarray(x[XB * c:XB * (c + 1)])
        in_maps.append(m)

    res = run_bass_kernel_spmd(nc, in_maps, list(range(N_CORES)))
    global _last_results
    _last_results = res.results
    out = np.concatenate([res.results[c]["y"] for c in range(N_CORES)], axis=0)
    return out.astype(np.float32)


def _build_in_maps(x, W1, b1, W2, b2, Wf, bf):
    D, NBLK, DOUT, N_CORES, B = (CFG["D"], CFG["NBLK"], CFG["DOUT"],
                                 CFG["N_CORES"], CFG["B"])
    XB = B // N_CORES
    peT, ones4, ident = _host_consts()
    consts = {
        "pe_t": peT,
        "w1t": np.ascontiguousarray(np.transpose(W1, (0, 2, 1))),
        "b1c": b1.reshape(NBLK, D, 1),
        "w2t": np.ascontiguousarray(np.transpose(W2, (0, 2, 1))),
        "b2c": b2.reshape(NBLK, D, 1),
        "wft": np.ascontiguousarray(Wf.T),
        "bfc": bf.reshape(DOUT, 1),
        "ones4": ones4,
        "ident_r": ident,
        "ident_b": ident.astype(ml_dtypes.bfloat16),
    }
    in_maps = []
    for c in range(N_CORES):
        m = dict(consts)
        m["x"] = np.ascontiguousarray(x[XB * c:XB * (c + 1)])
        in_maps.append(m)
    return in_maps


def time_device(inputs, iters=24, warm=4):
    """Estimate per-execution device time: async-dispatch slope with
    device-resident inputs and no buffer donation."""
    import jax
    from jax.sharding import Mesh, PartitionSpec
    from jax.experimental.shard_map import shard_map
    from concourse import bass2jax as b2j
    from concourse import mybir as mb

    n_cores = CFG["N_CORES"]
    nc = _prog_cache.get("prog")
    if nc is None:
        nc = _prog_cache["prog"] = build_program()
    b2j.install_neuronx_cc_hook()
    in_maps = _build_in_maps(**{k: np.asarray(v, np.float32)
                                for k, v in inputs.items()})

    partition_name = (nc.partition_id_tensor.name
                      if nc.partition_id_tensor else None)
    in_names, out_names, out_avals, zero_outs = [], [], [], []
    for alloc in nc.m.functions[0].allocations:
        if not isinstance(alloc, mb.MemoryLocationSet):
            continue
        name = alloc.memorylocations[0].name
        if alloc.kind == "ExternalInput":
            if name != partition_name:
                in_names.append(name)
        elif alloc.kind == "ExternalOutput":
            shape = tuple(alloc.tensor_shape)
            dtype = mb.dt.np(alloc.dtype)
            out_names.append(name)
            out_avals.append(jax.core.ShapedArray(shape, dtype))
            zero_outs.append(np.zeros(shape, dtype))
    n_params = len(in_names)
    all_names = list(in_names) + list(out_names)
    if partition_name is not None:
        all_names.append(partition_name)

    def _body(*args):
        operands = list(args)
        if partition_name is not None:
            operands.append(b2j.partition_id_tensor())
        outs = b2j._bass_exec_p.bind(
            *operands,
            out_avals=tuple(out_avals),
            in_names=tuple(all_names),
            out_names=tuple(out_names),
            lowering_input_output_aliases=(),
            sim_require_finite=True,
            sim_require_nnan=True,
            nc=nc,
        )
        return tuple(outs)

    devices = jax.devices()[:n_cores]
    mesh = Mesh(np.asarray(devices), ("core",))
    nin = n_params + len(out_names)
    sharded = jax.jit(
        shard_map(_body, mesh=mesh,
                  in_specs=(PartitionSpec("core"),) * nin,
                  out_specs=(PartitionSpec("core"),) * len(out_names),
                  check_rep=False),
        keep_unused=True,
    )
    per_core = [[np.asarray(in_maps[c][nm]) for nm in in_names]
                for c in range(n_cores)]
    concat_in = [np.concatenate([per_core[c][i] for c in range(n_cores)],
                                axis=0) for i in range(n_params)]
    concat_zeros = [np.zeros((n_cores * z.shape[0], *z.shape[1:]), z.dtype)
                    for z in zero_outs]
    args = [jax.device_put(a) for a in concat_in + concat_zeros]

    import time as _t
    for _ in range(warm):
        r = sharded(*args)
        jax.block_until_ready(r)

    def run_k(k):
        t0 = _t.perf_counter()
        rs = [sharded(*args) for _ in range(k)]
        jax.block_until_ready(rs)
        return _t.perf_counter() - t0

    k1, k2 = max(2, iters // 4), iters
    t_small = min(run_k(k1) for _ in range(3))
    t_big = min(run_k(k2) for _ in range(3))
    per = (t_big - t_small) / (k2 - k1)
    print(f"  [timing] {k1} iters: {t_small*1e3:.1f} ms, "
          f"{k2} iters: {t_big*1e3:.1f} ms -> slope {per*1e6:.0f} us")
    return per * 1e9
